# revision 1
# baseline (speedup 1.0000x reference)
"""Trainium2 Bass kernel for nn_MetaBaseline (global-cosine + DN4 few-shot scoring).

Math (per episode b):
  global: logits[q,k] = <qmean_hat, bmean_hat>          (means over the 5x5 spatial grid)
  DN4:    sim[q,p,k,l] = <q_patch[q,p], s_col_hat[k,l]>  -> sum of top-neighbor_k over l,
          summed over p, / neighbor_k
  out = r0 * logits + r1 * dn4

Device strategy (data-parallel, 8 episodes per NeuronCore):
  - host pre-normalizes the support side and appends the 5 class-mean columns:
    s_ext [640, 130] per episode; query laid out as q_mat [640, 1920] (qp-major,
    zero-padded from 1875); both bf16.
  - host normalizes the query patches too (q_hat), so the device does no scaling;
    the class-mean projections (cols 125:130) then carry a spurious 1/||q_patch||
    factor that the host-built A*||q_patch|| aggregation matrix undoes.
  - PE: sim_ext[qp, 0:130] = q_hat^T @ s_ext as 15 qp-tiles x 5 k-tiles of
    [128,128]x[128,130] bf16 matmuls; two qp-tiles share one fp32 PSUM bank
    [128,260] so each PSUM->SBUF copy (split between ACT and DVE) covers two.
  - DVE Max8 gives the top-8 of each 25-value support-patch group in one op;
    one strided reduce_sum of the first neighbor_k per episode gives the
    per-(patch,class) DN4 terms.
  - tiny matmuls against the aggregation matrices contract the 25 patches of
    each query across partitions (DN4 against the 0/1 matrix, globals against
    A*||q_patch||).
  - host applies 1/(25*||q_mean||), neighbor_k, and the r-weighted combine.
"""
import numpy as np
import ml_dtypes

N_CORES = 8
B, WAY, SHOT, D, H, W = 64, 5, 1, 640, 5, 5
NQ = 75
HW = H * W                 # 25
QP = NQ * HW               # 1875 query patches per episode
NT = 15                    # qp tiles of 128
QP_PAD = NT * 128          # 1920
ND = D // 128              # 5 contraction tiles
EPC = B // N_CORES         # 8 episodes per core
SCOLS = WAY * HW + WAY     # 130
GEPS = 1e-12               # eps of the global-cosine branch (torch F.normalize)

_CACHE = {}
_LAST_IN_MAPS = None


def _build(k: int):
    """Build + compile the SPMD NEFF for top-k = k (k <= 8)."""
    import concourse.bacc as bacc
    import concourse.mybir as mybir
    import concourse.tile as tile

    bf16 = mybir.dt.bfloat16
    f32 = mybir.dt.float32
    COPY = mybir.ActivationFunctionType.Copy

    nc = bacc.Bacc("TRN2", target_bir_lowering=False, debug=False)
    qm = nc.dram_tensor("qm", [EPC, ND, 128, QP_PAD], bf16, kind="ExternalInput")
    se = nc.dram_tensor("se", [ND, 128, EPC * SCOLS], bf16, kind="ExternalInput")
    amat = nc.dram_tensor("amat", [128, NT * NQ], bf16, kind="ExternalInput")
    am2 = nc.dram_tensor("am2", [128, EPC * NT * NQ], bf16, kind="ExternalInput")
    out = nc.dram_tensor("out", [EPC, WAY, 2 * NQ], f32, kind="ExternalOutput")

    with tile.TileContext(nc) as tc:
        with (
            tc.tile_pool(name="const", bufs=1) as cpool,
            tc.tile_pool(name="q", bufs=4 * ND) as qpool,
            tc.tile_pool(name="simps", bufs=4, space="PSUM") as simpool,
            tc.tile_pool(name="acc", bufs=2, space="PSUM") as accpool,
            tc.tile_pool(name="simsb", bufs=14) as sbpool,
            tc.tile_pool(name="out8", bufs=3) as o8pool,
            tc.tile_pool(name="draw", bufs=3) as drpool,
            tc.tile_pool(name="osb", bufs=2) as opool,
        ):
            sts = []
            for d in range(ND):
                st = cpool.tile([128, EPC * SCOLS], bf16, tag=f"se{d}")
                (nc.sync if d % 2 == 0 else nc.scalar).dma_start(st[:], se[d])
                sts.append(st)
            amat_t = cpool.tile([128, NT * NQ], bf16)
            am2_t = cpool.tile([128, EPC * NT * NQ], bf16)

            pending = []  # deferred tail: (e, draw, dn4_ps, glob_ps)

            def emit_tail():
                if not pending:
                    return
                e, draw, dn4_ps, glob_ps, simsbs = pending.pop()
                if e != EPC - 1:
                    for t in range(NT):
                        simsb, off = simsbs[t]
                        nc.tensor.matmul(
                            glob_ps[:], simsb[:, off + WAY * HW:off + SCOLS],
                            am2_t[:, (e * NT + t) * NQ:(e * NT + t + 1) * NQ],
                            start=(t == 0), stop=(t == NT - 1),
                        )
                for t in range(NT):
                    nc.tensor.matmul(
                        dn4_ps[:], draw[:, t * WAY:(t + 1) * WAY],
                        amat_t[:, t * NQ:(t + 1) * NQ],
                        start=(t == 0), stop=(t == NT - 1),
                    )
                osb = opool.tile([WAY, 2 * NQ], f32)
                nc.scalar.activation(osb[:, 0:NQ], dn4_ps[:], COPY)
                nc.scalar.activation(osb[:, NQ:2 * NQ], glob_ps[:], COPY)
                nc.sync.dma_start(out[e], osb[:])

            for e in range(EPC):
                qts = []
                for d in range(ND):
                    qt = qpool.tile([128, QP_PAD], bf16)
                    eng = nc.sync if d % 2 == 0 else nc.scalar
                    if e == 0:
                        eng.dma_start(qt[:, 0:256], qm[e, d, :, 0:256])
                    else:
                        eng.dma_start(qt[:], qm[e, d])
                    qts.append(qt)
                if e == 0:
                    for d in range(ND):
                        eng = nc.sync if d % 2 == 0 else nc.scalar
                        eng.dma_start(qts[d][:, 256:QP_PAD], qm[e, d, :, 256:QP_PAD])
                if e == 0:
                    # big constants ride behind the first episode's data
                    nc.sync.dma_start(amat_t[:], amat[:])
                    nc.scalar.dma_start(am2_t[:], am2[:])
                dn4_ps = accpool.tile([WAY, NQ], f32, tag="dn4ps")
                glob_ps = accpool.tile([WAY, NQ], f32, tag="globps")
                out8 = o8pool.tile([128, NT * WAY * 8], bf16)
                # tiles paired two-per-PSUM-bank: [0,1], [2,3], ..., [14]
                groups = [(2 * i, min(2 * i + 2, NT)) for i in range((NT + 1) // 2)]
                simsbs = {}
                for gi, (t0, t1) in enumerate(groups):
                    w = (t1 - t0) * SCOLS
                    simps = simpool.tile([128, 2 * SCOLS], f32, tag="simps")
                    for t in range(t0, t1):
                        off = (t - t0) * SCOLS
                        for d in range(ND):
                            nc.tensor.matmul(
                                simps[:, off:off + SCOLS],
                                qts[d][:, t * 128:(t + 1) * 128],
                                sts[d][:, e * SCOLS:(e + 1) * SCOLS],
                                start=(d == 0), stop=(d == ND - 1),
                            )
                    simsb = sbpool.tile([128, 2 * SCOLS], bf16)
                    for t in range(t0, t1):
                        off = (t - t0) * SCOLS
                        if gi == 0:
                            nc.vector.tensor_copy(
                                simsb[:, off:off + SCOLS], simps[:, off:off + SCOLS])
                        else:
                            nc.scalar.activation(
                                simsb[:, off:off + SCOLS], simps[:, off:off + SCOLS], COPY)
                        simsbs[t] = (simsb, off)
                        for kk in range(WAY):
                            g = t * WAY + kk
                            nc.vector.max(
                                out8[:, g * 8:(g + 1) * 8],
                                simsb[:, off + kk * HW:off + (kk + 1) * HW],
                            )
                    if gi == 1:
                        emit_tail()  # previous episode's aggregation matmuls
                    if e == EPC - 1:
                        for t in range(t0, t1):
                            simsb, off = simsbs[t]
                            nc.tensor.matmul(
                                glob_ps[:], simsb[:, off + WAY * HW:off + SCOLS],
                                am2_t[:, (e * NT + t) * NQ:(e * NT + t + 1) * NQ],
                                start=(t == 0), stop=(t == NT - 1),
                            )
                draw = drpool.tile([128, NT * WAY], bf16)
                o8v = out8[:].rearrange("p (g e) -> p g e", e=8)[:, :, 0:k]
                with nc.allow_low_precision("bf16 top-k sums feed a bf16 matmul"):
                    nc.vector.reduce_sum(draw[:], o8v, axis=mybir.AxisListType.X)
                pending.append((e, draw, dn4_ps, glob_ps, simsbs))
            emit_tail()
    nc.compile()
    return nc


def kernel(base, query, r, neighbor_k):
    from concourse.bass_utils import run_bass_kernel_spmd

    k = int(neighbor_k)
    assert 1 <= k <= 8, f"top-k must fit the Max8 output, got {k}"
    base = np.asarray(base, dtype=np.float32).reshape(B, WAY, D, HW)
    query = np.asarray(query, dtype=np.float32).reshape(B, NQ, D, HW)
    r = np.asarray(r, dtype=np.float32)

    # ---- host prep (layout + normalization metadata) ----
    # support: normalized columns + normalized class means -> s_ext [B, D, 130]
    s_norm = base / np.linalg.norm(base, axis=2, keepdims=True)
    bmean = base.mean(axis=3)                                     # [B, way, D]
    bm = bmean / np.maximum(
        np.linalg.norm(bmean, axis=2, keepdims=True), GEPS)
    s_ext = np.empty((B, D, SCOLS), dtype=np.float32)
    s_ext[:, :, :WAY * HW] = s_norm.transpose(0, 2, 1, 3).reshape(B, D, WAY * HW)
    s_ext[:, :, WAY * HW:] = bm.transpose(0, 2, 1)
    # [B, ND, 128, SCOLS] -> per-core [ND, 128, EPC*SCOLS]
    s_ext = s_ext.reshape(B, ND, 128, SCOLS).astype(ml_dtypes.bfloat16)
    s_ext = s_ext.reshape(N_CORES, EPC, ND, 128, SCOLS).transpose(0, 2, 3, 1, 4)
    s_ext = s_ext.reshape(N_CORES, ND, 128, EPC * SCOLS)

    # query: normalized patches, q_hat [B, D, 1920] (qp-major, zero-padded), bf16
    qn = np.sqrt(np.einsum("bqdp,bqdp->bqp", query, query))      # [B, nq, hw]
    q_hat = query / qn[:, :, None, :]
    q_mat = np.zeros((B, D, QP_PAD), dtype=ml_dtypes.bfloat16)
    q_mat[:, :, :QP] = q_hat.transpose(0, 2, 1, 3).reshape(B, D, QP)
    q_mat = q_mat.reshape(B, ND, 128, QP_PAD)
    qn_pad = np.zeros((B, QP_PAD), dtype=np.float32)
    qn_pad[:, :QP] = qn.reshape(B, QP)

    # query-mean norms for the global branch
    qmean = query.mean(axis=3)                                    # [B, nq, D]
    qmn = np.maximum(np.linalg.norm(qmean, axis=2), GEPS)         # [B, nq]

    # patch->query aggregation matrix (0/1), [128, NT*NQ]; and A*||q_patch||
    am = np.zeros((128, NT, NQ), dtype=np.float32)
    for t in range(NT):
        qp_idx = t * 128 + np.arange(128)
        valid = qp_idx < QP
        am[valid, t, qp_idx[valid] // HW] = 1.0
    am2 = am[None] * qn_pad.reshape(B, NT, 128).transpose(0, 2, 1)[:, :, :, None]
    am = am.reshape(128, NT * NQ).astype(ml_dtypes.bfloat16)
    am2 = am2.reshape(N_CORES, EPC, 128, NT * NQ).transpose(0, 2, 1, 3)
    am2 = np.ascontiguousarray(am2.reshape(N_CORES, 128, EPC * NT * NQ)).astype(ml_dtypes.bfloat16)

    if k not in _CACHE:
        _CACHE[k] = _build(k)
    nc = _CACHE[k]

    in_maps = []
    for c in range(N_CORES):
        sl = slice(c * EPC, (c + 1) * EPC)
        in_maps.append({
            "qm": np.ascontiguousarray(q_mat[sl]),
            "se": np.ascontiguousarray(s_ext[c]),
            "amat": am,
            "am2": am2[c],
        })
    global _LAST_IN_MAPS
    _LAST_IN_MAPS = in_maps
    res = run_bass_kernel_spmd(nc, in_maps, list(range(N_CORES)))
    dev = np.stack([res.results[c]["out"] for c in range(N_CORES)])  # [C, EPC, WAY, 150]
    dev = dev.reshape(B, WAY, 2 * NQ)

    dn4 = dev[:, :, :NQ].transpose(0, 2, 1) / k                   # [B, nq, way]
    glob = dev[:, :, NQ:].transpose(0, 2, 1) / (HW * qmn[:, :, None])
    return (r[0] * glob + r[1] * dn4).astype(np.float32)



# revision 6
# speedup vs baseline: 1.0129x; 1.0129x over previous
"""Trainium2 Bass kernel for nn_MetaBaseline (global-cosine + DN4 few-shot scoring).

Math (per episode b):
  global: logits[q,k] = <qmean_hat, bmean_hat>          (means over the 5x5 spatial grid)
  DN4:    sim[q,p,k,l] = <q_patch[q,p], s_col_hat[k,l]>  -> sum of top-neighbor_k over l,
          summed over p, / neighbor_k
  out = r0 * logits + r1 * dn4

Device strategy (data-parallel, 8 episodes per NeuronCore):
  - host pre-normalizes everything and folds the scalar weights in:
    support columns s_hat (125 per episode/d-tile), class means bm_hat*r0
    (5 extra cols), query patches q_hat, query means qm_hat; the DN4
    patch->query aggregation matrix amat carries r1/neighbor_k.
  - all device tensors are laid out partition-major per episode so each
    input is ONE contiguous dma_start per episode.
  - PE: per episode, 15 qp-tiles x 5 d-tiles of [128,128]x[128,125] bf16
    matmuls -> sim in PSUM (2 qp-tiles share a PSUM bank tile).
  - Scalar/GpSimd alternate on the paired PSUM->SBUF bf16 copies.
  - DVE Max8 per (qp-tile, way) gives top-8 of each 25-value group;
    GpSimd reduce_sum of the first neighbor_k -> draw [128qp, 75].
  - PE aggregation: one PSUM [way, nq] accumulates 15 DN4 matmuls
    (draw^T contracted against amat) plus 5 global matmuls
    (bm_hat*r0 contracted against qm_hat over d) -> final episode scores.
  - host just reshapes/transposes the f32 result.
"""
import numpy as np
import ml_dtypes

N_CORES = 8
B, WAY, SHOT, D, H, W = 64, 5, 1, 640, 5, 5
NQ = 75
HW = H * W                 # 25
QP = NQ * HW               # 1875 query patches per episode
NT = 15                    # qp tiles of 128
QP_PAD = NT * 128          # 1920
ND = D // 128              # 5 contraction tiles
EPC = B // N_CORES         # 8 episodes per core
SCOLS = WAY * HW + WAY     # 130 (125 support cols + 5 class means)
GEPS = 1e-12               # eps of the global-cosine branch (torch F.normalize)

_CACHE = {}
_LAST_IN_MAPS = None


def _build(k: int):
    """Build + compile the SPMD NEFF for top-k = k (k <= 8)."""
    import concourse.bacc as bacc
    import concourse.mybir as mybir
    import concourse.tile as tile

    bf16 = mybir.dt.bfloat16
    f32 = mybir.dt.float32
    COPY = mybir.ActivationFunctionType.Copy

    nc = bacc.Bacc("TRN2", target_bir_lowering=False, debug=False)
    qm = nc.dram_tensor("qm", [EPC, 128, NT * ND * 128], bf16, kind="ExternalInput")
    se = nc.dram_tensor("se", [EPC, 128, ND * SCOLS], bf16, kind="ExternalInput")
    qmh = nc.dram_tensor("qmh", [EPC, 128, ND * NQ], bf16, kind="ExternalInput")
    amat = nc.dram_tensor("amat", [128, NT * NQ], bf16, kind="ExternalInput")
    out = nc.dram_tensor("out", [EPC, WAY, NQ], f32, kind="ExternalOutput")

    with tile.TileContext(nc) as tc:
        with (
            tc.tile_pool(name="const", bufs=1) as cpool,
            tc.tile_pool(name="q", bufs=3) as qpool,
            tc.tile_pool(name="qmh", bufs=3) as qmhpool,
            tc.tile_pool(name="simps", bufs=4, space="PSUM") as simpool,
            tc.tile_pool(name="acc", bufs=2, space="PSUM") as accpool,
            tc.tile_pool(name="simsb", bufs=6) as sbpool,
            tc.tile_pool(name="out8", bufs=2) as o8pool,
            tc.tile_pool(name="draw", bufs=2) as drpool,
            tc.tile_pool(name="osb", bufs=2) as opool,
        ):
            se_t = cpool.tile([128, EPC * ND * SCOLS], bf16)
            amat_t = cpool.tile([128, NT * NQ], bf16)
            qts = [qpool.tile([128, NT * ND * 128], bf16, tag=f"qt{i}",
                              name=f"qt{i}") for i in range(3)]
            qmhs = [qmhpool.tile([128, ND * NQ], bf16, tag=f"qmh{i}",
                                 name=f"qmh{i}") for i in range(3)]

            W_EP = ND * SCOLS           # se cols per episode
            C_EP = NT * ND * 128        # qm cols per episode

            # ---- prologue DMAs: episode 0 arrives in fine chunks on many
            # queues so the first matmul chain starts within ~2us.
            nc.sync.dma_start(se_t[:, 0:W_EP], se[0])
            nc.scalar.dma_start(qts[0][:, 0:2 * ND * 128], qm[0, :, 0:2 * ND * 128])
            nc.gpsimd.dma_start(
                qts[0][:, 2 * ND * 128:6 * ND * 128], qm[0, :, 2 * ND * 128:6 * ND * 128])
            nc.sync.dma_start(
                qts[0][:, 6 * ND * 128:10 * ND * 128], qm[0, :, 6 * ND * 128:10 * ND * 128])
            nc.scalar.dma_start(
                qts[0][:, 10 * ND * 128:C_EP], qm[0, :, 10 * ND * 128:C_EP])
            nc.gpsimd.dma_start(qmhs[0][:], qmh[0])
            nc.gpsimd.dma_start(amat_t[:], amat[:])

            pending = []  # deferred tail: (e, draw_t, acc_ps)

            def emit_agg(e, draw_t):
                """DN4 + global aggregation matmuls for episode e -> one PSUM."""
                acc = accpool.tile([WAY, NQ], f32, tag="acc")
                for t in range(NT):
                    nc.tensor.matmul(
                        acc[:], draw_t[:, t * WAY:(t + 1) * WAY],
                        amat_t[:, t * NQ:(t + 1) * NQ],
                        start=(t == 0), stop=False,
                    )
                for d in range(ND):
                    off = (e * ND + d) * SCOLS
                    nc.tensor.matmul(
                        acc[:], se_t[:, off + WAY * HW:off + SCOLS],
                        qmhs[e % 3][:, d * NQ:(d + 1) * NQ],
                        start=False, stop=(d == ND - 1),
                    )
                osb = opool.tile([WAY, NQ], f32)
                nc.scalar.activation(osb[:], acc[:], COPY)
                nc.gpsimd.dma_start(out[e], osb[:])

            groups = [(2 * i, min(2 * i + 2, NT)) for i in range((NT + 1) // 2)]
            for e in range(EPC):
                qt = qts[e % 3]
                out8 = o8pool.tile([128, NT * WAY * 8], bf16)
                for gi, (t0, t1) in enumerate(groups):
                    simps = simpool.tile([128, 250], f32, tag="simps")
                    for t in range(t0, t1):
                        off = (t - t0) * WAY * HW
                        for d in range(ND):
                            nc.tensor.matmul(
                                simps[:, off:off + WAY * HW],
                                qt[:, (t * ND + d) * 128:(t * ND + d + 1) * 128],
                                se_t[:, (e * ND + d) * SCOLS:(e * ND + d) * SCOLS + WAY * HW],
                                start=(d == 0), stop=(d == ND - 1),
                            )
                    w = (t1 - t0) * WAY * HW
                    simsb = sbpool.tile([128, 250], bf16)
                    nc.scalar.activation(simsb[:, 0:w], simps[:, 0:w], COPY)
                    for t in range(t0, t1):
                        off = (t - t0) * WAY * HW
                        for kk in range(WAY):
                            g = t * WAY + kk
                            nc.vector.max(
                                out8[:, g * 8:(g + 1) * 8],
                                simsb[:, off + kk * HW:off + (kk + 1) * HW],
                            )
                    # prefetch + deferred aggregation, spread across the episode
                    if gi == 1:
                        if pending:
                            emit_agg(*pending.pop())
                    elif gi == 3:
                        if e + 1 < EPC:  # next episode's q, split over 2 queues
                            h = C_EP // 2
                            eng2 = nc.sync if e % 2 == 0 else nc.scalar
                            eng3 = nc.scalar if e % 2 == 0 else nc.sync
                            eng2.dma_start(qts[(e + 1) % 3][:, 0:h], qm[e + 1, :, 0:h])
                            eng3.dma_start(qts[(e + 1) % 3][:, h:C_EP], qm[e + 1, :, h:C_EP])
                    elif gi == 5:
                        if e + 1 < EPC:
                            nc.gpsimd.dma_start(
                                se_t[:, (e + 1) * W_EP:(e + 2) * W_EP], se[e + 1])
                            nc.gpsimd.dma_start(qmhs[(e + 1) % 3][:], qmh[e + 1])
                draw_t = drpool.tile([128, NT * WAY], bf16)
                o8v = out8[:].rearrange("p (g e) -> p g e", e=8)
                with nc.allow_low_precision("bf16 top-k sums feed a bf16 matmul"):
                    if k == 1:
                        nc.gpsimd.tensor_copy(draw_t[:], o8v[:, :, 0])
                    else:
                        nc.gpsimd.tensor_add(draw_t[:], o8v[:, :, 0], o8v[:, :, 1])
                        for j in range(2, k):
                            nc.gpsimd.tensor_add(draw_t[:], draw_t[:], o8v[:, :, j])
                pending.append((e, draw_t))
            emit_agg(*pending.pop())
    nc.compile()
    return nc


def kernel(base, query, r, neighbor_k):
    from concourse.bass_utils import run_bass_kernel_spmd

    k = int(neighbor_k)
    assert 1 <= k <= 8, f"top-k must fit the Max8 output, got {k}"
    base = np.asarray(base, dtype=np.float32).reshape(B, WAY, D, HW)
    query = np.asarray(query, dtype=np.float32).reshape(B, NQ, D, HW)
    r = np.asarray(r, dtype=np.float32)

    # ---- host prep (layout + normalization, scalar weights folded in) ----
    # support: normalized columns + r0-scaled normalized class means
    s_norm = base / np.linalg.norm(base, axis=2, keepdims=True)
    bmean = base.mean(axis=3)                                     # [B, way, D]
    bm = bmean / np.maximum(np.linalg.norm(bmean, axis=2, keepdims=True), GEPS)
    s_ext = np.empty((B, D, SCOLS), dtype=np.float32)
    s_ext[:, :, :WAY * HW] = s_norm.transpose(0, 2, 1, 3).reshape(B, D, WAY * HW)
    s_ext[:, :, WAY * HW:] = (r[0] * bm).transpose(0, 2, 1)
    # -> [B, 128, ND*SCOLS] partition-major (one dma per episode)
    s_ext = s_ext.reshape(B, ND, 128, SCOLS).transpose(0, 2, 1, 3)
    s_ext = np.ascontiguousarray(s_ext.reshape(B, 128, ND * SCOLS)).astype(ml_dtypes.bfloat16)

    # query patches: normalized, [B, 128, NT*ND*128] (tile-major free dim)
    qn = np.sqrt(np.einsum("bqdp,bqdp->bqp", query, query))      # [B, nq, hw]
    q_hat = query / qn[:, :, None, :]
    q_mat = np.zeros((B, D, QP_PAD), dtype=np.float32)
    q_mat[:, :, :QP] = q_hat.transpose(0, 2, 1, 3).reshape(B, D, QP)
    q_mat = q_mat.reshape(B, ND, 128, NT, 128).transpose(0, 2, 3, 1, 4)
    q_mat = np.ascontiguousarray(
        q_mat.reshape(B, 128, NT * ND * 128)).astype(ml_dtypes.bfloat16)

    # query means: normalized, [B, 128, ND*NQ]
    qmean = query.mean(axis=3)                                    # [B, nq, D]
    qmh = qmean / np.maximum(np.linalg.norm(qmean, axis=2, keepdims=True), GEPS)
    qmh = qmh.transpose(0, 2, 1).reshape(B, ND, 128, NQ).transpose(0, 2, 1, 3)
    qmh = np.ascontiguousarray(qmh.reshape(B, 128, ND * NQ)).astype(ml_dtypes.bfloat16)

    # patch->query aggregation matrix (r1/k folded), [128, NT*NQ]
    am = np.zeros((128, NT, NQ), dtype=np.float32)
    for t in range(NT):
        qp_idx = t * 128 + np.arange(128)
        valid = qp_idx < QP
        am[valid, t, qp_idx[valid] // HW] = r[1] / k
    am = am.reshape(128, NT * NQ).astype(ml_dtypes.bfloat16)

    if k not in _CACHE:
        _CACHE[k] = _build(k)
    nc = _CACHE[k]

    in_maps = []
    for c in range(N_CORES):
        sl = slice(c * EPC, (c + 1) * EPC)
        in_maps.append({
            "qm": q_mat[sl],
            "se": s_ext[sl],
            "qmh": qmh[sl],
            "amat": am,
        })
    global _LAST_IN_MAPS
    _LAST_IN_MAPS = in_maps
    res = run_bass_kernel_spmd(nc, in_maps, list(range(N_CORES)))
    dev = np.stack([res.results[c]["out"] for c in range(N_CORES)])  # [C, EPC, WAY, NQ]
    return np.ascontiguousarray(
        dev.reshape(B, WAY, NQ).transpose(0, 2, 1)).astype(np.float32)
